# revision 3
# baseline (speedup 1.0000x reference)
"""Trainium2 Bass kernel for BatchIrregularDownsample2d (D=2).

Contract: kernel(**inputs) takes the FULL inputs
    input:        [B, C, N]  float32
    pooling_mask: [B, 1, H, W] int32
and returns the FULL output [B, C, M] float32, where M is the max
per-batch compacted length (identical across batches for quadtree masks
with equal level histograms, which is what this module produces).

Strategy (pure data-parallel over B, one batch per NeuronCore):
  The reference gather G[b] splits into
    - an identity prefix  out[:, :start]            = in[:, :start]
    - a small gather      out[:, start:start+ng]    = in[:, start + rel[j]]
  where rel[j] < nelems = N - start fits in int16.

  Per core: DRAM->DRAM DMA for the prefix copy (one chunk per HWDGE
  queue). The gather source region [C=256, nelems] is loaded in slabs,
  cast+interleaved by the DVE into a bf16 buffer srcI[128, nelems, 2]
  holding both 128-partition C-chunks elementwise-interleaved (bf16 is
  within the 2e-2 harness tolerance; the identity prefix stays exact
  f32). srcI is double-buffered across iterations so the DVE fills
  buffer (k+1)%2 while a GPSIMD ap_gather with d=2 drains buffer k%2 —
  the ap_gather ucode's per-4-index-request cost is the kernel's floor,
  so everything else is hidden behind it. The gather runs as 2 calls
  (idx halves) so de-interleave+store of half 0 overlaps the gather of
  half 1. The DVE de-interleaves+upcasts each result plane into f32
  bounce buffers that are DMA'd out contiguously.
  Index arithmetic is host-side numpy (as in the original torch module,
  which syncs the mask to host anyway).
"""

import numpy as np

from concourse import bass, library_config, mybir
from concourse.bass_utils import run_bass_kernel_spmd

f32 = mybir.dt.float32
bf16 = mybir.dt.bfloat16
i16 = mybir.dt.int16

_NUM_CORES = 8
_N_SUB = 8    # gather-region load slabs (2 alternating staging slots)
_N_CALLS = 2  # gather calls per iteration (idx halves)


# ---------------------------------------------------------------------------
# Host-side index computation (replicates reference._build_indices, D=2)
# ---------------------------------------------------------------------------

def _batch_indices(mask2d):
    """mask2d: [H, W] int32 quadtree mask. Returns (start, rel_idx int64[ng])
    with absolute gather index = start + rel_idx."""
    D = 2
    s = 2 ** (D - 1)
    start = 0
    for i in range(D - 1):
        start += int((mask2d == i).sum()) // (4 ** i)
    cs = (mask2d >= D - 1)[::s, ::s]
    dt = (mask2d < D)[::s, ::s]
    r, c = np.nonzero(cs)
    topleft = ((r % 2) + (c % 2)) == 0
    dt_at = dt[r, c]
    keep_lower = topleft & ~dt_at
    pos = np.arange(r.shape[0])
    rel = np.concatenate([pos[dt_at], pos[keep_lower]]).astype(np.int64)
    return start, rel, int(r.shape[0])


def _halves(num_idxs):
    """Split the idx list into _N_CALLS 32-aligned positional pieces (the Q7
    ucode reads the index stream as 4-byte vectors, so every sub-gather's idx
    slice must start 4B-aligned = 32 idxs x 16 partitions)."""
    bounds = [i * (num_idxs // _N_CALLS) // 32 * 32 for i in range(_N_CALLS)]
    bounds.append(num_idxs)
    return [(bounds[i], bounds[i + 1]) for i in range(_N_CALLS)]


def _wrap_idxs(rel, num_idxs_pad):
    """Pack indices into the ap_gather layout: int16 [128, num_idxs_pad//16],
    index j at partition j%16, slot j//16, replicated across 8 Q7 groups."""
    padded = np.zeros(num_idxs_pad, np.int16)
    padded[: len(rel)] = rel
    wrapped = padded.reshape(num_idxs_pad // 16, 16).T  # [16, S]
    return np.tile(wrapped, (8, 1)).copy()  # [128, S]


def _make_idx_input(rel, num_idxs):
    """idxs input [128, num_idxs//16] (the halves are positional slices of
    the same list, so a single wrap covers all calls)."""
    return _wrap_idxs(rel, num_idxs)


def _slab_needs(rels, nelems, num_idxs):
    """Per gather call: how many load slabs its index values stay within
    (max over batches) — lets call h start before all slabs interleave."""
    E = (nelems + _N_SUB - 1) // _N_SUB
    needs = []
    for lo, hi in _halves(num_idxs):
        vmax = 0
        for rel in rels:
            seg = rel[lo : min(hi, len(rel))]
            if len(seg):
                vmax = max(vmax, int(seg.max()))
        needs.append(min(_N_SUB, max(1, -(-(vmax + 1) // E))))
    return tuple(needs)


# ---------------------------------------------------------------------------
# Bass program
# ---------------------------------------------------------------------------

_prog_cache = {}


def _build_program(C, N, start, ng, M, n_iters, needs,
                   parts=("copy", "load", "gather", "store")):
    """One batch per core: input [C, N] -> output [C, M].

    `needs[h]` = number of load slabs gather call h's indices stay within
    (host-computed upper bound; only used to start call h early).

    `parts` selects pipeline stages (for component benchmarking)."""
    key = (C, N, start, ng, M, n_iters, tuple(needs), tuple(parts))
    if key in _prog_cache:
        return _prog_cache[key]
    do_copy = "copy" in parts
    do_load = "load" in parts
    do_gather = "gather" in parts and do_load
    do_store = "store" in parts and do_gather

    assert C == 256, "kernel assumes two 128-partition C chunks"
    nelems = N - start                       # gather source region length
    num_idxs = ((ng + 31) // 32) * 32        # pad to %32 for ap_gather
    S = num_idxs // 16
    assert 0 < nelems <= 2 ** 15             # int16 cell addressing, bf16 d=2
    E = (nelems + _N_SUB - 1) // _N_SUB      # slab size
    slabs = [(e * E, min(nelems, (e + 1) * E)) for e in range(_N_SUB)]
    hs = _halves(num_idxs)
    assert len(needs) == _N_CALLS and all(1 <= n <= _N_SUB for n in needs)
    assert ng > hs[-1][0], "last call must contain real indices"

    nc = bass.Bass("TRN2")
    inp = nc.dram_tensor("input", [C, N], f32, kind="ExternalInput").ap()
    idxs = nc.dram_tensor("idxs", [128, S], i16, kind="ExternalInput").ap()
    out = nc.dram_tensor("output", [C, M], f32, kind="ExternalOutput").ap()

    # Staging slabs (slot parity): [*, 0, :] = chunk c0..127, [*, 1, :] =
    # chunk c128..255, so one DVE copy interleaves+casts both chunks.
    stg = [nc.alloc_sbuf_tensor(f"stg{i}", [128, 2, E], f32).ap() for i in range(2)]
    srcI = [
        nc.alloc_sbuf_tensor(f"srcI{i}", [128, nelems, 2], bf16).ap()
        for i in range(2)
    ]
    ogI = [
        nc.alloc_sbuf_tensor(f"ogI{h}", [128, hi - lo, 2], bf16).ap()
        for h, (lo, hi) in enumerate(hs)
    ]
    ogDe = [
        nc.alloc_sbuf_tensor(f"ogDe{h}", [128, hi - lo], f32).ap()
        for h, (lo, hi) in enumerate(hs)
    ]
    idxt = nc.alloc_sbuf_tensor("idxt", [128, S], i16).ap()

    K = n_iters
    from contextlib import ExitStack

    with ExitStack() as ctx:
        block = ctx.enter_context(nc.Block())
        se = [ctx.enter_context(nc.semaphore(f"se{i}")) for i in range(2)]
        sC = ctx.enter_context(nc.semaphore("sC"))     # prefix copies (+16)
        sI = ctx.enter_context(nc.semaphore("sI"))     # idx load (+16)
        # per-(call, plane) store sems (+16 each)
        sS = [
            [ctx.enter_context(nc.semaphore(f"sS{p}{h}")) for h in range(_N_CALLS)]
            for p in range(2)
        ]
        vI = ctx.enter_context(nc.semaphore("vI"))     # interleaves (+1, 8/iter)
        vD = ctx.enter_context(nc.semaphore("vD"))     # de-interleaves (+1, 4/iter)
        gp = ctx.enter_context(nc.semaphore("gp"))     # gathers (+1, 2/iter)

        @block.sync
        def _(sync):
            for k in range(K):
                if do_load:
                    for e, (lo, hi) in enumerate(slabs):
                        if do_gather:
                            # staging slot reused from slab e-2: its
                            # interleave copy must be done
                            sync.wait_ge(vI, max(0, _N_SUB * k + e - 1))
                            # self-wait for provably ordered slot updates
                            # (race-detector hygiene; implied by vI wait)
                            sync.wait_ge(
                                se[e % 2], 32 * (k * (_N_SUB // 2) + e // 2)
                            )
                        sync.dma_start(
                            out=stg[e % 2][:, 0, 0 : hi - lo],
                            in_=inp[0:128, start + lo : start + hi],
                        ).then_inc(se[e % 2], 16)
                        sync.dma_start(
                            out=stg[e % 2][:, 1, 0 : hi - lo],
                            in_=inp[128:256, start + lo : start + hi],
                        ).then_inc(se[e % 2], 16)
                if do_copy:
                    sync.dma_start(
                        out=out[0:128, 0:start], in_=inp[0:128, 0:start]
                    ).then_inc(sC, 16)
            if do_copy:
                sync.wait_ge(sC, 16 * K)
            if do_load and not do_gather:
                sync.wait_ge(se[0], 16 * K * _N_SUB)
                sync.wait_ge(se[1], 16 * K * _N_SUB)

        @block.vector
        def _(vec):
            if not do_gather:
                return

            def deinterleave(k, h):
                lo, hi = hs[h]
                vec.wait_ge(gp, _N_CALLS * k + h + 1)  # gather (k, h) done
                if do_store and k > 0:
                    # ogDe[h] last read by store (k-1, h, p=1)
                    vec.wait_ge(sS[1][h], 16 * k)
                vec.tensor_copy(
                    ogDe[h][:, 0 : hi - lo], ogI[h][:, :, 0]
                ).then_inc(vD, 1)
                if do_store:
                    # plane-1 overwrite: this iteration's plane-0 store done
                    vec.wait_ge(sS[0][h], 16 * (k + 1))
                vec.tensor_copy(
                    ogDe[h][:, 0 : hi - lo], ogI[h][:, :, 1]
                ).then_inc(vD, 1)

            for k in range(K):
                for e, (lo, hi) in enumerate(slabs):
                    n_uses = k * (_N_SUB // 2) + e // 2 + 1
                    vec.wait_ge(se[e % 2], 32 * n_uses)
                    if e == 0:
                        # srcI[k%2] overwrite: gathers of iter k-2 done
                        vec.wait_ge(gp, _N_CALLS * max(0, k - 1))
                    vec.tensor_copy(
                        srcI[k % 2][:, lo:hi, :],
                        stg[e % 2][:, :, 0 : hi - lo].rearrange("p c e -> p e c"),
                    ).then_inc(vI, 1)
                # de-interleaves of iteration k-1 emitted after this
                # iteration's interleaves: they only unblock once gather
                # (k-1, h) completes, by which time these interleaves have
                # run (both feed the DVE in order).
                if k > 0:
                    deinterleave(k - 1, 0)
                    deinterleave(k - 1, 1)
            deinterleave(K - 1, 0)
            deinterleave(K - 1, 1)

        @block.scalar
        def _(scalar):
            if do_gather:
                scalar.dma_start(out=idxt[:], in_=idxs[:]).then_inc(sI, 16)
            if do_copy:
                for k in range(K):
                    scalar.dma_start(
                        out=out[128:256, 0:start], in_=inp[128:256, 0:start]
                    ).then_inc(sC, 16)
                scalar.wait_ge(sC, 32 * K) if not do_store else None
            if do_store:
                for k in range(K):
                    for h in range(_N_CALLS):
                        lo, hi = hs[h]
                        real = min(hi, ng) - lo
                        for p in range(2):
                            scalar.wait_ge(vD, 4 * k + 2 * h + p + 1)
                            scalar.dma_start(
                                out=out[
                                    128 * p : 128 * (p + 1),
                                    start + lo : start + lo + real,
                                ],
                                in_=ogDe[h][:, 0:real],
                            ).then_inc(sS[p][h], 16)
                for p in range(2):
                    for h in range(_N_CALLS):
                        scalar.wait_ge(sS[p][h], 16 * K)

        @block.gpsimd
        def _(g):
            if not do_gather:
                return
            g.load_library(library_config.ap_gather)
            g.wait_ge(sI, 16)
            for k in range(K):
                for h in range(_N_CALLS):
                    lo, hi = hs[h]
                    # interleaves of iter k covering call h's sources done
                    g.wait_ge(vI, _N_SUB * k + needs[h])
                    if k > 0:
                        # ogI[h] reuse: its de-interleaves from k-1 done
                        g.wait_ge(vD, 4 * (k - 1) + 2 * (h + 1))
                    g.ap_gather(
                        out_ap=ogI[h][:],
                        in_ap=srcI[k % 2][:],
                        idxs_ap=idxt[:, lo // 16 : hi // 16],
                        channels=128,
                        num_elems=nelems,
                        d=2,
                        num_idxs=hi - lo,
                    ).then_inc(gp, 1)

    # Populate .instr bytes for extended-inst InstISA subclasses.
    mybir.codegen_inst_isa_subclasses(nc)

    _prog_cache[key] = (nc, num_idxs)
    return nc, num_idxs


# ---------------------------------------------------------------------------
# Public entry point
# ---------------------------------------------------------------------------

def kernel(input, pooling_mask, _n_iters=1):
    x = np.asarray(input)
    mask = np.asarray(pooling_mask)
    B, C, N = x.shape
    assert x.dtype == np.float32

    per_batch = [_batch_indices(mask[b, 0]) for b in range(B)]
    starts = {s for s, _, _ in per_batch}
    ngs = {len(r) for _, r, _ in per_batch}
    M = max(s + len(r) for s, r, _ in per_batch)

    start0 = per_batch[0][0]
    ng0 = len(per_batch[0][1])
    num_idxs0 = ((ng0 + 31) // 32) * 32
    device_ok = (
        len(starts) == 1
        and len(ngs) == 1
        and B == _NUM_CORES
        and C == 256
        and ng0 > 0
        and 0 < (N - start0) <= 2 ** 15
        and ng0 > _halves(num_idxs0)[-1][0]
    )
    if not device_ok:
        # Irregular shape structure across batches (not produced by this
        # module's mask builder) — fall back to a host gather.
        out = np.zeros((B, C, M), np.float32)
        for b, (s, rel, _) in enumerate(per_batch):
            n = s + len(rel)
            g = np.concatenate([np.arange(s, dtype=np.int64), s + rel])
            out[b, :, :n] = x[b][:, g]
        return out

    start = per_batch[0][0]
    ng = len(per_batch[0][1])
    rels = [r for _, r, _ in per_batch]
    needs = _slab_needs(rels, N - start, num_idxs0)

    nc, num_idxs = _build_program(C, N, start, ng, M, _n_iters, needs)
    in_maps = [
        {
            "input": np.ascontiguousarray(x[b]),
            "idxs": _make_idx_input(rels[b], num_idxs),
        }
        for b in range(B)
    ]
    res = run_bass_kernel_spmd(nc, in_maps, list(range(_NUM_CORES)))
    return np.stack([res.results[b]["output"] for b in range(B)])
